# revision 2
# baseline (speedup 1.0000x reference)
"""Trainium2 Bass kernel for 16-head attention (B=2, N=2048, D=1024) — v2.

Sharding: 8 cores = 2 batches x 4 head-groups (4 heads each = 2 pairs).

v2 redesign vs baseline (trace-driven):
  - The attention loop runs in (pair, n1-quarter of 512, n2-tile) steps.
    Both heads of a pair write scores into ONE [128,1024] f32 psum tile
    (head A cols 0-511, head B cols 512-1023, packed via tile_position),
    so a single [128,1024] ACTIVATE covers both heads and the scalar
    engine streams exps back-to-back (it is the roofline: ~171us/core).
  - Scores psum is ring-2 (double-buffered), so the next step's scores
    never wait on the current exp. attn@v accumulates [65,512] f32 per
    head over the 16 n2-tiles; the ones-column of vext yields the
    softmax denominator in row 64.
  - PSUM budget (8 banks): scores ring 2x[128,1024] (4) + attn
    accumulators 2x[65,512] (2) + mm ring 2x[128,512] (2) for the
    interleaved qk/v/proj matmuls.
  - Softmax denominators are DMA-transposed onto 128 partitions before
    the reciprocal (DVE iterative divide is 8 cyc/elem/lane: [1,512]
    recip was 4us; [128,4] is ~0.1us), transposed back, and broadcast
    via a DRAM bounce on otherwise-idle DMA queues.
  - proj is per-pair (host sums the two partial outputs), so pair0's
    projection runs mid-kernel instead of after pair1's divisions.
  - qk / v / proj matmuls are paced into the PE slack of the attention
    steps; input DMAs on gpsimd (SWDGE casts f32->fp16 in flight);
    outputs alternate sync/act HWDGE rings.
"""

import os
import sys

import numpy as np

sys.path.insert(0, "/opt/trn_rl_repo")

B, N, D = 2, 2048, 1024
NUM_HEADS = 16
HD = 64
N_CORES = 8
HEADS_PER_CORE = 4
HCOLS = HEADS_PER_CORE * HD  # 256
SCALE = HD ** -0.5

P = 128
KC = D // P  # 8 contraction chunks
NT = N // P  # 16 sequence tiles
NQ = 4  # n1 quarters of 512 per pair
STEPS = 2 * NQ * NT  # 128 global steps


def build_program(loop_n: int | str | None = None):
    import contextlib

    import concourse.bass as bass
    import concourse.tile as tile
    from concourse import bacc, mybir

    f32 = mybir.dt.float32
    f16 = mybir.dt.float16

    nc = bacc.Bacc()

    # inputs are pre-cast to fp16 host-side (make_in_maps): half the DMA
    # bytes and no SWDGE cast needed, so loads split across HWDGE queues
    xT_d = nc.dram_tensor("xT", [D, N], f16, kind="ExternalInput")
    wq_d = nc.dram_tensor("wq", [D, HCOLS], f16, kind="ExternalInput")
    wk_d = nc.dram_tensor("wk", [D, HCOLS], f16, kind="ExternalInput")
    wv_d = nc.dram_tensor("wv", [D, HCOLS], f16, kind="ExternalInput")
    wp_d = nc.dram_tensor("wp", [HCOLS, D], f16, kind="ExternalInput")
    out0_d = nc.dram_tensor("out0", [N, D], f16, kind="ExternalOutput")
    out1_d = nc.dram_tensor("out1", [N, D], f16, kind="ExternalOutput")
    den_d = nc.dram_tensor("dend", [8, 2, 512], f32)  # [instance, head, n1]
    rec_d = nc.dram_tensor("recd", [8, 2, 512], f32)
    niter_d = None
    if loop_n == "dyn":
        niter_d = nc.dram_tensor("niter", [1, 1], mybir.dt.int32, kind="ExternalInput")

    with tile.TileContext(nc) as tc:
        with (
            tc.tile_pool(name="persist", bufs=1) as persist,
            tc.tile_pool(name="work", bufs=2) as work,
            tc.tile_pool(name="psum", bufs=1, space="PSUM") as psum,
        ):
            if loop_n == "dyn":
                nt_sb = persist.tile([1, 1], mybir.dt.int32)
                nc.sync.dma_start(out=nt_sb, in_=niter_d[:])
                loop_bound = nc.values_load(nt_sb, min_val=0, max_val=4096)
            else:
                loop_bound = loop_n
            loop_cm = (
                tc.For_i(0, loop_bound, 1, hint_engines=(mybir.EngineType.PE,))
                if loop_n is not None
                else contextlib.nullcontext()
            )
            with loop_cm:
                # ---- persistent SBUF tiles ----
                xt = persist.tile([P, KC, N], f16)
                wq = persist.tile([P, KC, HCOLS], f16)
                wk = persist.tile([P, KC, HCOLS], f16)
                wv = persist.tile([P, KC, HCOLS], f16)
                wp = persist.tile([P, 2, D], f16)
                qT = persist.tile([P, 2, N], f16)
                kT = persist.tile([P, 2, N], f16)
                vext = persist.tile([P, HEADS_PER_CORE, NT * 65], f16)
                zbias = persist.tile([P, 1], f32)
                ones64 = persist.tile([P, 64], f32)
                denT = persist.tile([P, 8, 4], f32)  # [p, slot(=phase*2+head), c4]
                recT = persist.tile([P, 8, 4], f32)

                nc.vector.memset(zbias, 0.0)
                nc.vector.memset(ones64, 1.0)
                nc.vector.tensor_copy(
                    vext.rearrange("p h (t c) -> p h t c", c=65)[:, :, :, 64],
                    ones64.rearrange("p (h t) -> p h t", h=HEADS_PER_CORE),
                )

                # ---- input DMAs: fp16 copies. One big xt DMA (16 SDMA
                # engines split it internally; per-DMA completion latency
                # made 8 separate chunk DMAs land later). Weights on the
                # other queues.
                nc.scalar.dma_start(out=wk, in_=wk_d.rearrange("(c p) f -> p c f", p=P))
                nc.scalar.dma_start(out=wq, in_=wq_d.rearrange("(c p) f -> p c f", p=P))
                nc.sync.dma_start(out=xt, in_=xT_d.rearrange("(c p) f -> p c f", p=P))
                nc.gpsimd.dma_start(out=wv, in_=wv_d.rearrange("(c p) f -> p c f", p=P))
                nc.gpsimd.dma_start(out=wp, in_=wp_d.rearrange("(c p) f -> p c f", p=P))

                # attnout^T destinations: reuse wv (pair0) / wq (pair1) tiles
                # (dead after the v phase / qk phase respectively).
                ao = [
                    wv.rearrange("p c f -> p (c f)"),
                    wq.rearrange("p c f -> p (c f)"),
                ]
                out_ds = [out0_d, out1_d]

                # ---------- emission helpers ----------
                def emit_qk_chunk(hp, which, n1c):
                    """One [128,512] psum chunk of qT/kT for pair hp.
                    which: 0 = q, 1 = k. n1c in 0..3 (512-col chunks)."""
                    w = wq if which == 0 else wk
                    dst = qT if which == 0 else kT
                    ps = psum.tile([P, 512], f32, tag="mm", bufs=2, name="ps_qk")
                    for kc in range(KC):
                        nc.tensor.matmul(
                            ps,
                            w[:, kc, hp * P : (hp + 1) * P],
                            xt[:, kc, n1c * 512 : (n1c + 1) * 512],
                            start=(kc == 0),
                            stop=(kc == KC - 1),
                        )
                    nc.vector.tensor_copy(dst[:, hp, n1c * 512 : (n1c + 1) * 512], ps)

                def emit_v_tile(t2):
                    """v[n, hd] for all 4 heads, one 128-row n tile."""
                    psv = psum.tile([P, 512], f32, tag="mm", bufs=2, name="ps_v")
                    for kc in range(KC):
                        nc.tensor.matmul(
                            psv[:, :HCOLS],
                            xt[:, kc, t2 * P : (t2 + 1) * P],
                            wv[:, kc, :],
                            start=(kc == 0),
                            stop=(kc == KC - 1),
                        )
                    nc.vector.tensor_copy(
                        vext[:, :, t2 * 65 : t2 * 65 + 64],
                        psv[:, :HCOLS].rearrange("p (h d) -> p h d", h=HEADS_PER_CORE),
                    )

                def emit_scores(psc, hp, nq, t2):
                    """Packed scores^T for both heads of pair hp at n1
                    quarter nq, n2 tile t2. A -> cols 0-511, B -> 512-1023."""
                    ns = slice(nq * 512, (nq + 1) * 512)
                    nc.tensor.matmul(
                        psc[:, 0:512],
                        kT[0:64, hp, t2 * P : (t2 + 1) * P],
                        qT[0:64, hp, ns],
                        start=True,
                        stop=True,
                        tile_position=(0, 0),
                    )
                    nc.tensor.matmul(
                        psc[:, 512:1024],
                        kT[64:128, hp, t2 * P : (t2 + 1) * P],
                        qT[64:128, hp, ns],
                        start=True,
                        stop=True,
                        tile_position=(64, 0),
                    )

                def emit_attnv(ps_o, hp, a, t2, eAB):
                    h = 2 * hp + a
                    nc.tensor.matmul(
                        ps_o,
                        vext[:, h, t2 * 65 : t2 * 65 + 65],
                        eAB[:, a * 512 : (a + 1) * 512],
                        start=(t2 == 0),
                        stop=(t2 == NT - 1),
                    )

                def emit_division(inst, hp, nq, ps_pair):
                    """Divide both heads' [65,512] accumulators by their
                    denominators; write into ao[hp] cols nq*512..+512."""
                    cs = slice(nq * 512, (nq + 1) * 512)
                    ph = inst % 4
                    oexts = []
                    # all four DMA hops of the chain go on gpsimd: FIFO order
                    # within the queue removes the cross-queue sem waits, and
                    # gpsimd is idle mid-window (inputs done early).
                    for a in (0, 1):
                        oext = work.tile([65, 512], f32, tag="oext", bufs=4, name="oext")
                        nc.vector.tensor_copy(oext, ps_pair[a])
                        oexts.append(oext)
                        nc.gpsimd.dma_start(out=den_d[inst, a, :], in_=oext[64:65, :])
                    for a in (0, 1):
                        srcp = den_d[inst, a, :]
                        tr = bass.AP(
                            tensor=srcp.tensor,
                            offset=srcp.offset,
                            ap=[[1, P], [P, 4]],
                        )
                        nc.gpsimd.dma_start(out=denT[:, 2 * ph + a, :], in_=tr)
                    nc.vector.reciprocal(
                        recT[:, 2 * ph : 2 * ph + 2, :],
                        denT[:, 2 * ph : 2 * ph + 2, :],
                    )
                    for a in (0, 1):
                        dstp = rec_d[inst, a, :]
                        tr = bass.AP(
                            tensor=dstp.tensor,
                            offset=dstp.offset,
                            ap=[[1, P], [P, 4]],
                        )
                        nc.gpsimd.dma_start(out=tr, in_=recT[:, 2 * ph + a, :])
                    for a in (0, 1):
                        lb = work.tile([64, 512], f32, tag="lb", bufs=4, name="lb")
                        srcp = rec_d[inst, a, :]
                        bc = bass.AP(
                            tensor=srcp.tensor,
                            offset=srcp.offset,
                            ap=[[0, 64]] + [list(dd) for dd in srcp.ap],
                        )
                        nc.gpsimd.dma_start(out=lb, in_=bc)
                        nc.vector.tensor_mul(
                            ao[hp][64 * a : 64 * a + 64, cs], oexts[a][0:64, :], lb
                        )

                def emit_proj_chunk(hp, nt):
                    """Partial projection for pair hp, one 128-row n1 tile."""
                    osb = work.tile([P, D], f16, tag="osb", bufs=3, name="osb")
                    for jc in range(2):
                        pj = psum.tile([P, 512], f32, tag="mm", bufs=2, name="ps_pj")
                        nc.tensor.matmul(
                            pj,
                            ao[hp][:, nt * P : (nt + 1) * P],
                            wp[:, hp, jc * 512 : (jc + 1) * 512],
                            start=True,
                            stop=True,
                        )
                        nc.vector.tensor_copy(osb[:, jc * 512 : (jc + 1) * 512], pj)
                    eng = nc.sync if nt % 2 == 0 else nc.scalar
                    eng.dma_start(out=out_ds[hp][nt * P : (nt + 1) * P, :], in_=osb)

                # ---------- extras pacing (PE slack fillers) ----------
                extras = {s: [] for s in range(STEPS)}

                def add_extra(s, fn):
                    extras[min(max(s, 0), STEPS - 1)].append(fn)

                # v tiles 1..15 one step ahead of first use (tile 0 in ramp)
                for t2 in range(1, NT):
                    add_extra(t2 - 1, (lambda t=t2: emit_v_tile(t)))
                # kT chunks 1-3 of pair0 (chunk c covers t2=4c..4c+3)
                add_extra(1, lambda: emit_qk_chunk(0, 1, 1))
                add_extra(5, lambda: emit_qk_chunk(0, 1, 2))
                add_extra(9, lambda: emit_qk_chunk(0, 1, 3))
                # qT chunks 1-3 of pair0 (chunk c needed at step 16c)
                add_extra(13, lambda: emit_qk_chunk(0, 0, 1))
                add_extra(17, lambda: emit_qk_chunk(0, 0, 2))
                add_extra(21, lambda: emit_qk_chunk(0, 0, 3))
                # pair1 (needed from step 64; k chunk c by step 64+4c,
                # q chunk c by step 64+16c)
                _p1 = [(1, 1, 0), (1, 1, 1), (1, 0, 0), (1, 1, 2), (1, 1, 3), (1, 0, 1)]
                for j, (hp_, w_, c_) in enumerate(_p1):
                    add_extra(25 + 6 * j, (lambda a=hp_, b=w_, c=c_: emit_qk_chunk(a, b, c)))
                # qT chunks 2,3 of pair1: must be emitted before pair1's
                # first division (step 79) which overwrites the wq tile
                # region (ao[1]); needed only at steps 96/112.
                add_extra(65, lambda: emit_qk_chunk(1, 0, 2))
                add_extra(71, lambda: emit_qk_chunk(1, 0, 3))

                # proj chunks enter the PE queue only PROJ_DELAY steps after
                # their division was emitted: the division's DMA round-trips
                # (~8us) must complete first, or the in-order PE queue stalls
                # behind the waiting proj matmul.
                PROJ_DELAY = 16
                proj_ready = []  # (earliest_step, closure)
                inst_counter = [0]

                def division_for(s, hp, nq, ps_pair):
                    inst = inst_counter[0]
                    inst_counter[0] += 1
                    emit_division(inst, hp, nq, ps_pair)
                    for nt in range(nq * 4, nq * 4 + 4):
                        proj_ready.append(
                            (s + PROJ_DELAY, lambda a=hp, b=nt: emit_proj_chunk(a, b))
                        )

                # ---- ramp: just enough for step 0 ----
                emit_qk_chunk(0, 1, 0)
                emit_qk_chunk(0, 0, 0)
                emit_v_tile(0)

                # ---- software-pipelined attention loop ----
                def step_of(s):
                    hp, r = divmod(s, NQ * NT)
                    nq, t2 = divmod(r, NT)
                    return hp, nq, t2

                psc_cur = psum.tile([P, 1024], f32, tag="sc", bufs=2, name="psc")
                emit_scores(psc_cur, *step_of(0))
                ps_o_cur = None

                for s in range(STEPS):
                    hp, nq, t2 = step_of(s)
                    if t2 == 0:
                        ps_o_cur = [
                            psum.tile([65, 512], f32, tag="po", bufs=2, name="ps_oA"),
                            psum.tile([65, 512], f32, tag="po", bufs=2, name="ps_oB"),
                        ]
                    # exp reads psc_cur (emitted before next scores: WAR order)
                    eAB = work.tile([P, 1024], f16, tag="expS", bufs=4, name="eAB")
                    nc.scalar.activation(
                        eAB, psc_cur, bass.mybir.ActivationFunctionType.Exp,
                        bias=zbias, scale=SCALE,
                    )
                    # next step's scores into the other ring slot
                    if s + 1 < STEPS:
                        psc_next = psum.tile([P, 1024], f32, tag="sc", bufs=2, name="psc")
                        emit_scores(psc_next, *step_of(s + 1))
                        psc_cur = psc_next
                    # attn@v for this step
                    emit_attnv(ps_o_cur[0], hp, 0, t2, eAB)
                    emit_attnv(ps_o_cur[1], hp, 1, t2, eAB)
                    # paced extras + at most one matured proj chunk per step
                    for fn in extras[s]:
                        fn()
                    if proj_ready and proj_ready[0][0] <= s:
                        proj_ready.pop(0)[1]()
                    if t2 == NT - 1:
                        division_for(s, hp, nq, ps_o_cur)

                while proj_ready:
                    proj_ready.pop(0)[1]()

    nc.finalize()
    return nc


def make_in_maps(x, w_qk, w_v, w_proj):
    in_maps = []
    xTb = [np.ascontiguousarray(x[b].T) for b in range(B)]
    for c in range(N_CORES):
        b, g = divmod(c, N_CORES // B)
        h0 = g * HCOLS
        in_maps.append(
            {
                "xT": np.ascontiguousarray(xTb[b], dtype=np.float16),
                "wq": np.ascontiguousarray(w_qk[:, h0 : h0 + HCOLS], dtype=np.float16),
                "wk": np.ascontiguousarray(
                    w_qk[:, D + h0 : D + h0 + HCOLS], dtype=np.float16
                ),
                "wv": np.ascontiguousarray(w_v[:, h0 : h0 + HCOLS], dtype=np.float16),
                "wp": np.ascontiguousarray(w_proj[h0 : h0 + HCOLS, :], dtype=np.float16),
            }
        )
    return in_maps


def combine_results(results, b_proj):
    gpb = N_CORES // B
    out = np.empty((B, N, D), dtype=np.float32)
    for b in range(B):
        acc = None
        for g in range(gpb):
            r = results[b * gpb + g]
            part = r["out0"].astype(np.float32) + r["out1"].astype(np.float32)
            acc = part if acc is None else acc + part
        out[b] = acc + b_proj[None, :]
    return out


# revision 3
# speedup vs baseline: 1.0037x; 1.0037x over previous
"""Trainium2 Bass kernel for 16-head attention (B=2, N=2048, D=1024) — v2.

Sharding: 8 cores = 2 batches x 4 head-groups (4 heads each = 2 pairs).

v2 redesign vs baseline (trace-driven):
  - The attention loop runs in (pair, n1-quarter of 512, n2-tile) steps.
    Both heads of a pair write scores into ONE [128,1024] f32 psum tile
    (head A cols 0-511, head B cols 512-1023, packed via tile_position),
    so a single [128,1024] ACTIVATE covers both heads and the scalar
    engine streams exps back-to-back (it is the roofline: ~171us/core).
  - Scores psum is ring-2 (double-buffered), so the next step's scores
    never wait on the current exp. attn@v accumulates [65,512] f32 per
    head over the 16 n2-tiles; the ones-column of vext yields the
    softmax denominator in row 64.
  - PSUM budget (8 banks): scores ring 2x[128,1024] (4) + attn
    accumulators 2x[65,512] (2) + mm ring 2x[128,512] (2) for the
    interleaved qk/v/proj matmuls.
  - Softmax denominators are DMA-transposed onto 128 partitions before
    the reciprocal (DVE iterative divide is 8 cyc/elem/lane: [1,512]
    recip was 4us; [128,4] is ~0.1us), transposed back, and broadcast
    via a DRAM bounce on otherwise-idle DMA queues.
  - proj is per-pair (host sums the two partial outputs), so pair0's
    projection runs mid-kernel instead of after pair1's divisions.
  - qk / v / proj matmuls are paced into the PE slack of the attention
    steps; input DMAs on gpsimd (SWDGE casts f32->fp16 in flight);
    outputs alternate sync/act HWDGE rings.
"""

import os
import sys

import numpy as np

sys.path.insert(0, "/opt/trn_rl_repo")

B, N, D = 2, 2048, 1024
NUM_HEADS = 16
HD = 64
N_CORES = 8
HEADS_PER_CORE = 4
HCOLS = HEADS_PER_CORE * HD  # 256
SCALE = HD ** -0.5

P = 128
KC = D // P  # 8 contraction chunks
NT = N // P  # 16 sequence tiles
NQ = 4  # n1 quarters of 512 per pair
STEPS = 2 * NQ * NT  # 128 global steps


def build_program(loop_n: int | str | None = None):
    import contextlib

    import concourse.bass as bass
    import concourse.tile as tile
    from concourse import bacc, mybir

    f32 = mybir.dt.float32
    f16 = mybir.dt.float16

    nc = bacc.Bacc()

    # inputs are pre-cast to fp16 host-side (make_in_maps): half the DMA
    # bytes and no SWDGE cast needed, so loads split across HWDGE queues
    xT_d = nc.dram_tensor("xT", [D, N], f16, kind="ExternalInput")
    wq_d = nc.dram_tensor("wq", [D, HCOLS], f16, kind="ExternalInput")
    wk_d = nc.dram_tensor("wk", [D, HCOLS], f16, kind="ExternalInput")
    wv_d = nc.dram_tensor("wv", [D, HCOLS], f16, kind="ExternalInput")
    wp_d = nc.dram_tensor("wp", [HCOLS, D], f16, kind="ExternalInput")
    out0_d = nc.dram_tensor("out0", [N, D], f16, kind="ExternalOutput")
    out1_d = nc.dram_tensor("out1", [N, D], f16, kind="ExternalOutput")
    den_d = nc.dram_tensor("dend", [8, 2, 512], f32)  # [instance, head, n1]
    rec_d = nc.dram_tensor("recd", [8, 2, 512], f32)
    niter_d = None
    if loop_n == "dyn":
        niter_d = nc.dram_tensor("niter", [1, 1], mybir.dt.int32, kind="ExternalInput")

    with tile.TileContext(nc) as tc:
        with (
            tc.tile_pool(name="persist", bufs=1) as persist,
            tc.tile_pool(name="work", bufs=2) as work,
            tc.tile_pool(name="psum", bufs=1, space="PSUM") as psum,
        ):
            if loop_n == "dyn":
                nt_sb = persist.tile([1, 1], mybir.dt.int32)
                nc.sync.dma_start(out=nt_sb, in_=niter_d[:])
                loop_bound = nc.values_load(nt_sb, min_val=0, max_val=4096)
            else:
                loop_bound = loop_n
            loop_cm = (
                tc.For_i(0, loop_bound, 1, hint_engines=(mybir.EngineType.PE,))
                if loop_n is not None
                else contextlib.nullcontext()
            )
            with loop_cm:
                # ---- persistent SBUF tiles ----
                xt = persist.tile([P, KC, N], f16)
                wq = persist.tile([P, KC, HCOLS], f16)
                wk = persist.tile([P, KC, HCOLS], f16)
                wv = persist.tile([P, KC, HCOLS], f16)
                wp = persist.tile([P, 2, D], f16)
                qT = persist.tile([P, 2, N], f16)
                kT = persist.tile([P, 2, N], f16)
                vext = persist.tile([P, HEADS_PER_CORE, NT * 65], f16)
                zbias = persist.tile([P, 1], f32)
                ones64 = persist.tile([P, 64], f32)
                denT = persist.tile([P, 8, 4], f32)  # [p, slot(=phase*2+head), c4]
                recT = persist.tile([P, 8, 4], f32)

                nc.vector.memset(zbias, 0.0)
                nc.vector.memset(ones64, 1.0)
                nc.vector.tensor_copy(
                    vext.rearrange("p h (t c) -> p h t c", c=65)[:, :, :, 64],
                    ones64.rearrange("p (h t) -> p h t", h=HEADS_PER_CORE),
                )

                # ---- input DMAs: fp16 copies. One big xt DMA (16 SDMA
                # engines split it internally; per-DMA completion latency
                # made 8 separate chunk DMAs land later). Weights on the
                # other queues.
                nc.scalar.dma_start(out=wk, in_=wk_d.rearrange("(c p) f -> p c f", p=P))
                nc.scalar.dma_start(out=wq, in_=wq_d.rearrange("(c p) f -> p c f", p=P))
                nc.sync.dma_start(out=xt, in_=xT_d.rearrange("(c p) f -> p c f", p=P))
                nc.gpsimd.dma_start(out=wv, in_=wv_d.rearrange("(c p) f -> p c f", p=P))
                nc.gpsimd.dma_start(out=wp, in_=wp_d.rearrange("(c p) f -> p c f", p=P))

                # attnout^T destinations: reuse wv (pair0) / wq (pair1) tiles
                # (dead after the v phase / qk phase respectively).
                ao = [
                    wv.rearrange("p c f -> p (c f)"),
                    wq.rearrange("p c f -> p (c f)"),
                ]
                out_ds = [out0_d, out1_d]

                # ---------- emission helpers ----------
                def emit_qk_chunk(hp, which, n1c):
                    """One [128,512] psum chunk of qT/kT for pair hp.
                    which: 0 = q, 1 = k. n1c in 0..3 (512-col chunks)."""
                    w = wq if which == 0 else wk
                    dst = qT if which == 0 else kT
                    ps = psum.tile([P, 512], f32, tag="mm", bufs=2, name="ps_qk")
                    for kc in range(KC):
                        nc.tensor.matmul(
                            ps,
                            w[:, kc, hp * P : (hp + 1) * P],
                            xt[:, kc, n1c * 512 : (n1c + 1) * 512],
                            start=(kc == 0),
                            stop=(kc == KC - 1),
                        )
                    nc.vector.tensor_copy(dst[:, hp, n1c * 512 : (n1c + 1) * 512], ps)

                def emit_v_tile(t2):
                    """v[n, hd] for all 4 heads, one 128-row n tile."""
                    psv = psum.tile([P, 512], f32, tag="mm", bufs=2, name="ps_v")
                    for kc in range(KC):
                        nc.tensor.matmul(
                            psv[:, :HCOLS],
                            xt[:, kc, t2 * P : (t2 + 1) * P],
                            wv[:, kc, :],
                            start=(kc == 0),
                            stop=(kc == KC - 1),
                        )
                    nc.vector.tensor_copy(
                        vext[:, :, t2 * 65 : t2 * 65 + 64],
                        psv[:, :HCOLS].rearrange("p (h d) -> p h d", h=HEADS_PER_CORE),
                    )

                def emit_scores(psc, hp, nq, t2):
                    """Packed scores^T for both heads of pair hp at n1
                    quarter nq, n2 tile t2. A -> cols 0-511, B -> 512-1023."""
                    ns = slice(nq * 512, (nq + 1) * 512)
                    nc.tensor.matmul(
                        psc[:, 0:512],
                        kT[0:64, hp, t2 * P : (t2 + 1) * P],
                        qT[0:64, hp, ns],
                        start=True,
                        stop=True,
                        tile_position=(0, 0),
                    )
                    nc.tensor.matmul(
                        psc[:, 512:1024],
                        kT[64:128, hp, t2 * P : (t2 + 1) * P],
                        qT[64:128, hp, ns],
                        start=True,
                        stop=True,
                        tile_position=(64, 0),
                    )

                def emit_attnv(ps_o, hp, a, t2, eAB):
                    h = 2 * hp + a
                    nc.tensor.matmul(
                        ps_o,
                        vext[:, h, t2 * 65 : t2 * 65 + 65],
                        eAB[:, a * 512 : (a + 1) * 512],
                        start=(t2 == 0),
                        stop=(t2 == NT - 1),
                    )

                def emit_division(inst, hp, nq, ps_pair):
                    """Divide both heads' [65,512] accumulators by their
                    denominators; write into ao[hp] cols nq*512..+512."""
                    cs = slice(nq * 512, (nq + 1) * 512)
                    ph = inst % 4
                    oexts = []
                    # all four DMA hops of the chain go on gpsimd: FIFO order
                    # within the queue removes the cross-queue sem waits, and
                    # gpsimd is idle mid-window (inputs done early).
                    for a in (0, 1):
                        oext = work.tile([65, 512], f32, tag="oext", bufs=4, name="oext")
                        nc.vector.tensor_copy(oext, ps_pair[a])
                        oexts.append(oext)
                        nc.gpsimd.dma_start(out=den_d[inst, a, :], in_=oext[64:65, :])
                    # spread den across 128 partitions in CONTIGUOUS 4-elem
                    # chunks (partition p holds den[4p..4p+3]): the recip only
                    # needs lane parallelism, not a true transpose, and the
                    # chunked layout keeps every DMA hop contiguous (the
                    # element-granular gather/scatter APs took ~8us each to
                    # complete).
                    for a in (0, 1):
                        nc.gpsimd.dma_start(
                            out=denT[:, 2 * ph + a, :],
                            in_=den_d[inst, a, :].rearrange("(p c) -> p c", p=P),
                        )
                    nc.vector.reciprocal(
                        recT[:, 2 * ph : 2 * ph + 2, :],
                        denT[:, 2 * ph : 2 * ph + 2, :],
                    )
                    for a in (0, 1):
                        nc.gpsimd.dma_start(
                            out=rec_d[inst, a, :].rearrange("(p c) -> p c", p=P),
                            in_=recT[:, 2 * ph + a, :],
                        )
                    for a in (0, 1):
                        lb = work.tile([64, 512], f32, tag="lb", bufs=4, name="lb")
                        srcp = rec_d[inst, a, :]
                        bc = bass.AP(
                            tensor=srcp.tensor,
                            offset=srcp.offset,
                            ap=[[0, 64]] + [list(dd) for dd in srcp.ap],
                        )
                        nc.gpsimd.dma_start(out=lb, in_=bc)
                        nc.vector.tensor_mul(
                            ao[hp][64 * a : 64 * a + 64, cs], oexts[a][0:64, :], lb
                        )

                def emit_proj_chunk(hp, nt):
                    """Partial projection for pair hp, one 128-row n1 tile."""
                    osb = work.tile([P, D], f16, tag="osb", bufs=3, name="osb")
                    for jc in range(2):
                        pj = psum.tile([P, 512], f32, tag="mm", bufs=2, name="ps_pj")
                        nc.tensor.matmul(
                            pj,
                            ao[hp][:, nt * P : (nt + 1) * P],
                            wp[:, hp, jc * 512 : (jc + 1) * 512],
                            start=True,
                            stop=True,
                        )
                        nc.vector.tensor_copy(osb[:, jc * 512 : (jc + 1) * 512], pj)
                    eng = nc.sync if nt % 2 == 0 else nc.scalar
                    eng.dma_start(out=out_ds[hp][nt * P : (nt + 1) * P, :], in_=osb)

                # ---------- extras pacing (PE slack fillers) ----------
                extras = {s: [] for s in range(STEPS)}

                def add_extra(s, fn):
                    extras[min(max(s, 0), STEPS - 1)].append(fn)

                # v tiles 1..15 one step ahead of first use (tile 0 in ramp)
                for t2 in range(1, NT):
                    add_extra(t2 - 1, (lambda t=t2: emit_v_tile(t)))
                # kT chunks 1-3 of pair0 (chunk c covers t2=4c..4c+3)
                add_extra(1, lambda: emit_qk_chunk(0, 1, 1))
                add_extra(5, lambda: emit_qk_chunk(0, 1, 2))
                add_extra(9, lambda: emit_qk_chunk(0, 1, 3))
                # qT chunks 1-3 of pair0 (chunk c needed at step 16c)
                add_extra(13, lambda: emit_qk_chunk(0, 0, 1))
                add_extra(17, lambda: emit_qk_chunk(0, 0, 2))
                add_extra(21, lambda: emit_qk_chunk(0, 0, 3))
                # pair1 (needed from step 64; k chunk c by step 64+4c,
                # q chunk c by step 64+16c)
                _p1 = [(1, 1, 0), (1, 1, 1), (1, 0, 0), (1, 1, 2), (1, 1, 3), (1, 0, 1)]
                for j, (hp_, w_, c_) in enumerate(_p1):
                    add_extra(25 + 6 * j, (lambda a=hp_, b=w_, c=c_: emit_qk_chunk(a, b, c)))
                # qT chunks 2,3 of pair1: must be emitted before pair1's
                # first division (step 79) which overwrites the wq tile
                # region (ao[1]); needed only at steps 96/112.
                add_extra(65, lambda: emit_qk_chunk(1, 0, 2))
                add_extra(71, lambda: emit_qk_chunk(1, 0, 3))

                # proj chunks enter the PE queue only PROJ_DELAY steps after
                # their division was emitted: the division's DMA round-trips
                # (~8us) must complete first, or the in-order PE queue stalls
                # behind the waiting proj matmul.
                PROJ_DELAY = 16
                # no proj pops before all qk extras are done: qk and proj
                # share the 2-slot mm psum ring, so a qk LDWEIGHTS right
                # after a proj allocation chains onto the proj's (division-
                # gated) drain and blocks the in-order PE queue.
                PROJ_START = 56
                proj_ready = []  # (earliest_step, closure)
                inst_counter = [0]

                def division_for(s, hp, nq, ps_pair):
                    inst = inst_counter[0]
                    inst_counter[0] += 1
                    emit_division(inst, hp, nq, ps_pair)
                    for nt in range(nq * 4, nq * 4 + 4):
                        proj_ready.append(
                            (s + PROJ_DELAY, lambda a=hp, b=nt: emit_proj_chunk(a, b))
                        )

                # ---- ramp: just enough for step 0 ----
                emit_qk_chunk(0, 1, 0)
                emit_qk_chunk(0, 0, 0)
                emit_v_tile(0)

                # ---- software-pipelined attention loop ----
                def step_of(s):
                    hp, r = divmod(s, NQ * NT)
                    nq, t2 = divmod(r, NT)
                    return hp, nq, t2

                psc_cur = psum.tile([P, 1024], f32, tag="sc", bufs=2, name="psc")
                emit_scores(psc_cur, *step_of(0))
                ps_o_cur = None

                for s in range(STEPS):
                    hp, nq, t2 = step_of(s)
                    if t2 == 0:
                        ps_o_cur = [
                            psum.tile([65, 512], f32, tag="po", bufs=2, name="ps_oA"),
                            psum.tile([65, 512], f32, tag="po", bufs=2, name="ps_oB"),
                        ]
                    # exp reads psc_cur (emitted before next scores: WAR order)
                    eAB = work.tile([P, 1024], f16, tag="expS", bufs=4, name="eAB")
                    nc.scalar.activation(
                        eAB, psc_cur, bass.mybir.ActivationFunctionType.Exp,
                        bias=zbias, scale=SCALE,
                    )
                    # next step's scores into the other ring slot
                    if s + 1 < STEPS:
                        psc_next = psum.tile([P, 1024], f32, tag="sc", bufs=2, name="psc")
                        emit_scores(psc_next, *step_of(s + 1))
                        psc_cur = psc_next
                    # attn@v for this step
                    emit_attnv(ps_o_cur[0], hp, 0, t2, eAB)
                    emit_attnv(ps_o_cur[1], hp, 1, t2, eAB)
                    # paced extras + at most one matured proj chunk per step
                    for fn in extras[s]:
                        fn()
                    npop = 0 if s < PROJ_START else (1 if s < 100 else 2)
                    while npop and proj_ready and proj_ready[0][0] <= s:
                        proj_ready.pop(0)[1]()
                        npop -= 1
                    if t2 == NT - 1:
                        division_for(s, hp, nq, ps_o_cur)

                while proj_ready:
                    proj_ready.pop(0)[1]()

    nc.finalize()
    return nc


def make_in_maps(x, w_qk, w_v, w_proj):
    in_maps = []
    xTb = [np.ascontiguousarray(x[b].T) for b in range(B)]
    for c in range(N_CORES):
        b, g = divmod(c, N_CORES // B)
        h0 = g * HCOLS
        in_maps.append(
            {
                "xT": np.ascontiguousarray(xTb[b], dtype=np.float16),
                "wq": np.ascontiguousarray(w_qk[:, h0 : h0 + HCOLS], dtype=np.float16),
                "wk": np.ascontiguousarray(
                    w_qk[:, D + h0 : D + h0 + HCOLS], dtype=np.float16
                ),
                "wv": np.ascontiguousarray(w_v[:, h0 : h0 + HCOLS], dtype=np.float16),
                "wp": np.ascontiguousarray(w_proj[h0 : h0 + HCOLS, :], dtype=np.float16),
            }
        )
    return in_maps


def combine_results(results, b_proj):
    gpb = N_CORES // B
    out = np.empty((B, N, D), dtype=np.float32)
    for b in range(B):
        acc = None
        for g in range(gpb):
            r = results[b * gpb + g]
            part = r["out0"].astype(np.float32) + r["out1"].astype(np.float32)
            acc = part if acc is None else acc + part
        out[b] = acc + b_proj[None, :]
    return out
